# revision 10
# baseline (speedup 1.0000x reference)
# Mixture-of-two-experts (modality-routed) token GEMM on 8 Trainium2 NeuronCores.
#
# reference computes BOTH expert GEMMs and selects per token; only one GEMM per
# token is needed. Strategy (expert-dispatch, per the sharding hint):
#   host: partition tokens by type_id. Device capacity is exactly 8192 tokens
#         per expert (4 cores x 2048); the few overflow tokens of the heavier
#         expert (binomial imbalance, ~tens of tokens) are computed on host
#         with BLAS during the gather/scatter phase (MoE capacity spill). This
#         keeps every core at exactly 16 output tiles - the 128-token x 4-core
#         quantization would otherwise force 17 tiles (+6.25% compute).
#   device (SPMD, uniform program): plain GEMM  y[tok, e] = x[tok, :] @ Wt + b
#         with fp16 operands, fp32 PSUM accumulation. Cores 0-3 carry expert-0
#         tokens + W0, cores 4-7 expert-1 tokens + W1 (weights arrive as data,
#         so the per-core program is identical).
#   host: inverse-scatter per-expert outputs back to [B, S, D] fp32.

import os
import sys
import time

import numpy as np

for _p in ("/opt/trn_rl_repo", "/root/.axon_site/_ro/trn_rl_repo"):
    if os.path.isdir(_p) and _p not in sys.path:
        sys.path.insert(0, _p)

import concourse.bacc as bacc
import concourse.mybir as mybir
import concourse.tile as tile
from concourse.bass_utils import run_bass_kernel_spmd

D = 2048
KT = D // 128  # 16 contraction tiles
N_CORES = 8
CORES_PER_EXPERT = 4
N_TOK = 2048  # tokens per core: 16 output tiles, exact 8-core balance
SPILL_MAX = 2048  # max tokens/expert computed host-side (imbalance overflow)

_PROGRAM_CACHE: dict[int, object] = {}
LAST_RESULTS = None  # BassKernelResults of the most recent launch (for profiling)


def _build_program(n_tok: int):
    """One NeuronCore program: y[n_tok, D] = xt.T @ wt + bias (fp16 in, fp32 out)."""
    m_tiles = n_tok // 128
    f16 = mybir.dt.float16
    f32 = mybir.dt.float32

    nc = bacc.Bacc("TRN2", target_bir_lowering=False, debug=False, num_devices=N_CORES)
    xt = nc.dram_tensor("xt", [KT, 128, n_tok], f16, kind="ExternalInput").ap()
    wt = nc.dram_tensor("wt", [KT, 128, D], f16, kind="ExternalInput").ap()
    bias = nc.dram_tensor("bias", [128, D], f32, kind="ExternalInput").ap()
    # fp16 output: results are fp32 in PSUM; the DVE bias-add rounds to fp16
    # on its SBUF write. Halves the output DMA traffic and the final-drain
    # tail; adds ~3e-4 relative error (budget is 2e-2). Host upcasts.
    y = nc.dram_tensor("y", [n_tok, D], f16, kind="ExternalOutput").ap()
    y_t = y.rearrange("(m p) e -> m p e", p=128)

    # The PE can only keep 2 full-width PSUM accumulation chains in flight, so
    # during the operand-load ramp it would starve between k-tile arrivals.
    # Fix: the first N_SPLIT m-tiles accumulate k=0..7 into SBUF partials as
    # soon as the first half of the k-tiles lands (phase A), and finish
    # k=8..15 later (phase B). Everything else runs the plain full-k walk.
    # n_split=4 sizes the per-k (weight, x-head) DMA pair at 640 KB = 1.79us,
    # just under the 2-chain PE demand of ~1.73us/k-tile, so the ramp runs
    # nearly gap-free, while phase A+B still provide enough head-only work
    # (~55us) to cover the full operand load (~47us).
    n_split = 4 if m_tiles >= 8 else 0

    with tile.TileContext(nc) as tc:
        with (
            tc.tile_pool(name="wp", bufs=1) as wp,
            tc.tile_pool(name="xp", bufs=1) as xp,
            tc.tile_pool(name="bp", bufs=1) as bp,
            tc.tile_pool(name="ap", bufs=1) as apool,
            tc.tile_pool(name="op", bufs=3) as op_,
            tc.tile_pool(name="pp", bufs=2, space="PSUM") as pp,
        ):
            # Whole operand set fits in SBUF (~200 KiB/partition with the
            # partial accumulators); per-k tiles so matmuls start as soon as
            # the first slices land. DMA instruction count is kept low (each
            # cross-engine dep edge costs ~130ns in the end-of-kernel
            # semaphore-reset storm): only the first two k-tiles get
            # fine-grained transfers (they gate the PE start), the rest are
            # batched. Single HWDGE ring: FIFO transfer order doubles as the
            # priority scheme - (w, x-head) pairs first, then bias, tails
            # last.
            head = n_split * 128
            tail = n_tok - head
            xh, wk = [], []
            bias_s = bp.tile([128, D], f32, name="bias_s")
            for k in range(KT):
                ws = wp.tile([128, D], f16, name=f"w{k}", tag=f"w{k}")
                h = None
                if n_split:
                    h = xp.tile([128, head], f16, name=f"xh{k}", tag=f"xh{k}")
                if k < 2:
                    # finer arrival granularity during the DMA ramp: matmuls
                    # on the first two output chunks can start before the
                    # full weight tile lands (Tile deps are range-based).
                    # xh0 rides between the w0 halves so the very first
                    # chain's chunk-0/1 matmuls are unblocked earliest.
                    nc.sync.dma_start(ws[:, 0 : D // 2], wt[k][:, 0 : D // 2])
                    if h is not None:
                        nc.sync.dma_start(h[:], xt[k][:, 0:head])
                    nc.sync.dma_start(ws[:, D // 2 : D], wt[k][:, D // 2 : D])
                else:
                    nc.sync.dma_start(ws[:], wt[k])
                    # per-k x-head transfer right behind its weight tile:
                    # HWDGE FIFO order = arrival order, so each (w, xh) pair
                    # lands just as the phase-A/B chains need that k-tile
                    if h is not None:
                        nc.sync.dma_start(h[:], xt[k][:, 0:head])
                wk.append(ws)
                if h is not None:
                    xh.append(h)
            nc.sync.dma_start(bias_s[:], bias[:])
            # tails: only needed by the late full-k walks; batch 8 k-tiles
            # per transfer to cut issue count
            xtl = []
            for g, (k0, k1) in enumerate(((0, 8), (8, KT))):
                t = xp.tile([128, k1 - k0, tail], f16, name=f"xt{g}", tag=f"xt{g}")
                nc.sync.dma_start(
                    t[:], xt[k0:k1][:, :, head:n_tok].rearrange("k p n -> p k n")
                )
                xtl.append(t)

            def lhs_slice(k, m):
                if m < n_split:
                    return xh[k][:, m * 128 : (m + 1) * 128]
                j = m - n_split
                t = xtl[0] if k < 8 else xtl[1]
                return t[:, k % 8, j * 128 : (j + 1) * 128]

            # PE warm-up: matmuls on a zeroed tile, no DMA dependency. Runs
            # during the DMA ramp (PE would idle anyway) and flips the HAM
            # clock gate to 8/8 before the first real matmul. ~8 cold matmuls
            # span the ~3.4us HAM window; more would delay the first real MM.
            wz = bp.tile([128, 512], f16, name="wz")
            nc.gpsimd.memset(wz[:], 0.0)
            psw = pp.tile([128, 512], f32, name="psw", tag="ps")
            for _ in range(8):
                nc.tensor.matmul(psw[:], wz[:, 0:128], wz[:], start=True, stop=True)

            def mm_chain(ps, m, ks):
                first = last = None
                for j, k in enumerate(ks):
                    lhsT = lhs_slice(k, m)  # [K, M] stationary
                    for c in range(4):
                        mm = nc.tensor.matmul(
                            ps[:, c * 512 : (c + 1) * 512],
                            lhsT,
                            wk[k][:, c * 512 : (c + 1) * 512],
                            start=(j == 0),
                            stop=(j == len(ks) - 1),
                        )
                        first = first or mm
                        last = mm
                return first, last

            def drain(ps, addend, m):
                # single full-width op: DVE reads PSUM across banks fine, and
                # fewer instructions -> fewer sems -> shorter end-of-kernel
                # semaphore-reset storm
                ot = op_.tile([128, D], f16, name=f"ot{m}", tag="ot")
                nc.vector.tensor_add(ot[:], ps[:], addend[:])
                nc.sync.dma_start(y_t[m], ot[:])

            prev_last = None

            def pin(first, reason):
                # keep the PE stream in emission order chain-by-chain: the
                # scheduler otherwise hoists later chains (gated on late k
                # arrivals) ahead of ready work and stalls the PE
                if prev_last is not None:
                    tile.add_dep_helper(
                        first.ins, prev_last.ins, sync=False, reason=reason
                    )

            # Pins enforce PHASE order only (all A before any B before any F):
            # chains WITHIN a phase stay unpinned so the scheduler can
            # interleave them - during the DMA ramp each arriving k-pair then
            # unlocks work from every in-flight chain, not just one.
            acc = {}
            a_lasts = []
            for m in range(n_split):  # phase A: k=0..7 -> SBUF partial
                ps = pp.tile([128, D], f32, name=f"psa{m}", tag="ps")
                fa, la = mm_chain(ps, m, range(KT // 2))
                a_lasts.append(la)
                # no bias here: bias arrives after the (w, x-head) pairs and
                # must not gate the phase-A psum drains
                a = apool.tile([128, D], f32, name=f"acc{m}", tag=f"acc{m}")
                nc.vector.tensor_copy(a[:], ps[:])
                acc[m] = a

            b_lasts = []
            for m in range(n_split):  # phase B: k=8..15 + partial + bias
                ps = pp.tile([128, D], f32, name=f"psb{m}", tag="ps")
                fb, lb = mm_chain(ps, m, range(KT // 2, KT))
                for la in a_lasts:
                    tile.add_dep_helper(fb.ins, la.ins, sync=False, reason="A->B")
                b_lasts.append(lb)
                ot = op_.tile([128, D], f16, name=f"otb{m}", tag="ot")
                nc.vector.tensor_add(ot[:], ps[:], acc[m][:])
                nc.vector.tensor_add(ot[:], ot[:], bias_s[:])
                nc.sync.dma_start(y_t[m], ot[:])
            prev_last = None

            full = list(range(n_split, m_tiles))
            for m in full[:-1]:
                ps = pp.tile([128, D], f32, name=f"ps{m}", tag="ps")
                ff, lf = mm_chain(ps, m, range(KT))
                if m == full[0]:
                    for lb in b_lasts:
                        tile.add_dep_helper(ff.ins, lb.ins, sync=False, reason="B->F")
                else:
                    pin(ff, f"chain order F{m}")
                prev_last = lf
                drain(ps, bias_s, m)

            # last m-tile: accumulate one 512-wide chunk at a time so chunk c
            # drains (DVE add + DMA) while chunk c+1's matmuls still run -
            # shortens the kernel tail after the final matmul. Each chunk gets
            # its OWN psum tile (same tag -> alternates the 2 slots): chunks
            # in one tile serialize ~800ns each on accumulation-group
            # tracking, separate tiles pipeline cleanly. The final 512 chunk
            # drains in two 256 halves: the very last HBM write (and its ~2us
            # completion receipt, which the kernel end waits on) then covers
            # 64KB instead of 256KB.
            m = full[-1]
            ot = op_.tile([128, D], f16, name=f"ot{m}", tag="ot")
            bounds = (0, 512, 1024, 1536, 1920, 2048)
            for c in range(5):
                sl = slice(bounds[c], bounds[c + 1])
                w_c = bounds[c + 1] - bounds[c]
                ps = pp.tile([128, w_c], f32, name=f"psl{c}", tag="ps")
                for j, k in enumerate(range(KT)):
                    mm = nc.tensor.matmul(
                        ps[:],
                        lhs_slice(k, m),
                        wk[k][:, sl],
                        start=(j == 0),
                        stop=(j == KT - 1),
                    )
                    if j == 0 and c == 0:
                        pin(mm, f"chain order F{m}")
                nc.vector.tensor_add(ot[:, sl], ps[:], bias_s[:, sl])
                nc.sync.dma_start(y_t[m][:, sl], ot[:, sl])

    nc.compile()
    return nc


def _get_program(n_tok: int):
    if n_tok not in _PROGRAM_CACHE:
        _PROGRAM_CACHE[n_tok] = _build_program(n_tok)
    return _PROGRAM_CACHE[n_tok]


def kernel(hidden_states, type_ids, W0, b0, W1, b1, _trace=False, _tmpdir=None):
    global LAST_RESULTS

    B, S, D_ = hidden_states.shape
    assert D_ == D
    x = np.ascontiguousarray(np.asarray(hidden_states, dtype=np.float32)).reshape(
        B * S, D
    )
    t = np.asarray(type_ids).reshape(B * S)

    idx = [np.nonzero(t == e)[0] for e in (0, 1)]
    counts = [len(i) for i in idx]
    # Device capacity: 8192 tokens/expert/launch (4 cores x 2048, 16 output
    # tiles each). Tokens beyond capacity (expert imbalance overflow, or an
    # extremely skewed split) run host-side up to SPILL_MAX; beyond that,
    # re-launch the same program over further token slices.
    n_tok = N_TOK
    cap = n_tok * CORES_PER_EXPERT
    n_launches = max(1, -(-(max(counts) - SPILL_MAX) // cap))
    dev_counts = [min(c, cap * n_launches) for c in counts]

    nc = _get_program(n_tok)

    w_np = [np.asarray(W, dtype=np.float32) for W in (W0, W1)]
    b_np = [np.asarray(b, dtype=np.float32) for b in (b0, b1)]
    wts, biases = [], []
    for W, b in zip(w_np, b_np):
        wts.append(
            np.ascontiguousarray(W.T.astype(np.float16)).reshape(KT, 128, D)
        )
        biases.append(np.ascontiguousarray(np.broadcast_to(b, (128, D))))

    gathered = [x[idx[e]].astype(np.float16) for e in (0, 1)]  # [count_e, D]

    out = np.empty((B * S, D), dtype=np.float32)
    parts = [[], []]
    for li in range(n_launches):
        in_maps = []
        for e in (0, 1):
            g = gathered[e][li * cap : min((li + 1) * cap, dev_counts[e])]
            if g.shape[0] < cap:
                g = np.concatenate(
                    [g, np.zeros((cap - g.shape[0], D), np.float16)], axis=0
                )
            for c in range(CORES_PER_EXPERT):
                chunk = g[c * n_tok : (c + 1) * n_tok]  # [n_tok, D]
                xt_c = np.ascontiguousarray(chunk.T).reshape(KT, 128, n_tok)
                in_maps.append({"xt": xt_c, "wt": wts[e], "bias": biases[e]})

        res = None
        for attempt in range(3):
            try:
                res = run_bass_kernel_spmd(
                    nc, in_maps, list(range(N_CORES)), trace=_trace, tmpdir=_tmpdir
                )
                break
            except Exception:
                # transient NRT_EXEC_UNIT_UNRECOVERABLE has been observed when
                # a run starts right as a previous process tears the device down
                if attempt == 2:
                    raise
                time.sleep(10)
        LAST_RESULTS = res
        for e in (0, 1):
            parts[e].extend(
                res.results[e * CORES_PER_EXPERT + c]["y"]
                for c in range(CORES_PER_EXPERT)
            )

    for e in (0, 1):
        out[idx[e][: dev_counts[e]]] = np.concatenate(parts[e], axis=0)[
            : dev_counts[e]
        ]
        if dev_counts[e] < counts[e]:  # host-side capacity spill (fp32 BLAS)
            spill_idx = idx[e][dev_counts[e] :]
            out[spill_idx] = x[spill_idx] @ w_np[e].T + b_np[e]
    return out.reshape(B, S, D)
